# revision 4
# baseline (speedup 1.0000x reference)
"""Sharded kNN retrieval kernel for Trainium2 (8 NeuronCores).

Strategy (v2, fp8 + grouped top-8):
  - Host: l2-normalize queries; queries bf16, img_memory fp8(e4m3, x16 scale);
    per-core transposed layout memt[c] = [128(d-part), 4(d-block), 25000(rows)].
  - Device (SPMD x8), per 2048-col block:
      PE:  sim = qT.T @ memT  (bf16 x fp8 -> PSUM f32)             ~2.1us
      DVE: 16:1 group max (tensor_reduce) straight off PSUM,        ~2.4us
           then MAX8 + FIND_INDEX8 over the 128 group maxes
      DMA: 1.05 MB fp8 per block                                    ~2.6us
    Stages pipeline; each core streams its 12.8 MB shard once.
  - Host: expand group candidates (x16 rows), rank by approx cos, exact f32
    rescore of the top rows; containment + duplicate-index checks with exact
    block-recompute fallback; assemble the reference output exactly.
"""

import numpy as np
import ml_dtypes

import concourse.bass as bass
import concourse.tile as tile
import concourse.mybir as mybir
from concourse import bass_utils

BF16 = ml_dtypes.bfloat16
FP8 = ml_dtypes.float8_e4m3

B = 128
D = 512
N = 200000
NCORES = 8
NSHARD = N // NCORES          # 25000
K = 3
ID_THRESHOLD = 0.15
SOFT_SCALE = 5.0
MSCALE = np.float32(16.0)     # img_memory scaled by 16 before fp8 quantization

WBLK = 2048
_full = NSHARD // WBLK        # 12
_rem = NSHARD - _full * WBLK  # 424
# (base, width, group_width)
BLOCKS = [(j * WBLK, WBLK, 16) for j in range(_full)] + \
         ([(NSHARD - _rem, _rem, 8)] if _rem else [])
NBLK = len(BLOCKS)            # 13

_NC_CACHE = {}


def _build_nc():
    if "nc" in _NC_CACHE:
        return _NC_CACHE["nc"]
    nc = bass.Bass("TRN2", target_bir_lowering=False, debug=False, num_devices=NCORES)
    qt = nc.dram_tensor("qt", [128, 4, 128], mybir.dt.bfloat16, kind="ExternalInput")
    # packed: block j occupies cols [4*base, 4*base+4*w) with b-major sublayout
    memt = nc.dram_tensor("memt", [128, 4 * NSHARD], mybir.dt.float8e4, kind="ExternalInput")
    vals_out = nc.dram_tensor("vals", [128, NBLK * 8], mybir.dt.float32, kind="ExternalOutput")
    idx_out = nc.dram_tensor("idx", [128, NBLK * 8], mybir.dt.uint32, kind="ExternalOutput")

    with tile.TileContext(nc) as tc:
        with (
            tc.tile_pool(name="qt_pool", bufs=1) as qt_pool,
            tc.tile_pool(name="mem_pool", bufs=3) as mem_pool,
            tc.tile_pool(name="red_pool", bufs=2) as red_pool,
            tc.tile_pool(name="res_pool", bufs=1) as res_pool,
            tc.tile_pool(name="psum_pool", bufs=2, space="PSUM") as psum_pool,
        ):
            qt_tile = qt_pool.tile([128, 4, 128], mybir.dt.bfloat16)
            nc.sync.dma_start(qt_tile[:], qt[:])

            vals_tile = res_pool.tile([128, NBLK * 8], mybir.dt.float32)
            idx_tile = res_pool.tile([128, NBLK * 8], mybir.dt.uint32)

            for j, (base, w, gw) in enumerate(BLOCKS):
                g = w // gw
                mt = mem_pool.tile([128, 4 * w], mybir.dt.float8e4, tag="mem")
                nc.sync.dma_start(mt[:], memt[:, 4 * base:4 * base + 4 * w])
                ps = psum_pool.tile([128, g, gw], mybir.dt.float32, tag="ps")
                for s0 in range(0, w, 512):
                    sw = min(512, w - s0)
                    for b in range(4):
                        nc.tensor.matmul(
                            ps[:, s0 // gw:(s0 + sw) // gw, :],
                            qt_tile[:, b, :],
                            mt[:, b * w + s0:b * w + s0 + sw],
                            start=(b == 0),
                            stop=(b == 3),
                        )
                red = red_pool.tile([128, g], mybir.dt.float32, tag="red")
                nc.vector.tensor_reduce(red[:], ps[:], axis=mybir.AxisListType.X,
                                        op=mybir.AluOpType.max)
                nc.vector.max(vals_tile[:, j * 8:(j + 1) * 8], red[:])
                nc.vector.max_index(idx_tile[:, j * 8:(j + 1) * 8],
                                    vals_tile[:, j * 8:(j + 1) * 8], red[:])

            nc.sync.dma_start(vals_out[:], vals_tile[:])
            nc.sync.dma_start(idx_out[:], idx_tile[:])
    _split_excess_waits(nc)
    _NC_CACHE["nc"] = nc
    return nc


def _split_excess_waits(nc, keep=1):
    """Walrus's MM instruction struct fits only one embedded sync wait; move
    extra waits emitted by Tile onto standalone NoOps just before the MM."""
    ctr = 0
    for fn in nc.m.functions:
        for blk in fn.blocks:
            newl = []
            for inst in blk.instructions:
                si = inst.sync_info
                if (type(inst).__name__ != "InstNoOp" and si is not None
                        and si.on_wait and len(si.on_wait) > keep):
                    waits = list(si.on_wait)
                    for w in waits[:-keep]:
                        nop = mybir.InstNoOp(name=f"I-waitnop-{ctr}")
                        ctr += 1
                        nop.engine = inst.engine
                        nop.sync_info = mybir.SyncInfo(on_wait=[w], on_update=[])
                        newl.append(nop)
                    inst.sync_info = mybir.SyncInfo(
                        on_wait=waits[-keep:], on_update=list(si.on_update or []))
                newl.append(inst)
            blk.instructions = newl


def run_device_topk(qt_host, memt_cores, trace=False):
    """Run the SPMD device kernel.  Returns (vals [8,128,NBLK*8] f32 raw-dot,
    idx [8,128,NBLK*8] uint32 group indices, BassKernelResults)."""
    nc = _build_nc()
    in_maps = [{"qt": qt_host, "memt": memt_cores[c]} for c in range(NCORES)]
    res = bass_utils.run_bass_kernel_spmd(
        nc, in_maps, core_ids=list(range(NCORES)), trace=trace,
    )
    vals = np.stack([res.results[c]["vals"] for c in range(NCORES)]) / MSCALE
    idx = np.stack([res.results[c]["idx"] for c in range(NCORES)])
    return vals, idx, res


def _prep_inputs(i_feats, img_memory):
    qn = i_feats / np.linalg.norm(i_feats, axis=1, keepdims=True)
    qn = qn.astype(np.float32)
    qn_bf = qn.astype(BF16)
    qt_host = np.ascontiguousarray(qn_bf.reshape(B, 4, 128).transpose(2, 1, 0))

    def _pack(c):
        shard8 = (img_memory[c * NSHARD:(c + 1) * NSHARD] * MSCALE).astype(FP8)
        segs = []
        for base, w, _ in BLOCKS:
            seg = shard8[base:base + w].reshape(w, 4, 128).transpose(2, 1, 0)
            segs.append(np.ascontiguousarray(seg).reshape(128, 4 * w))
        return np.concatenate(segs, axis=1)

    from concurrent.futures import ThreadPoolExecutor
    with ThreadPoolExecutor(max_workers=NCORES) as ex:
        memt_cores = list(ex.map(_pack, range(NCORES)))
    return qn, qn_bf, qt_host, memt_cores


# max |device_raw/16 - exact_raw| bound: fp8 quant (~0.13 observed max) +
# bf16 query rounding (2^-8*||m|| ~ 0.09) + f32 accum slack.
DELTA_RAW = np.float32(0.35)


def _exact_topk(qn, img_memory, vals, idx, mnorm):
    """Global exact top-(K+1) per query from device group candidates.

    Device reports top-8 *groups* per block: group g covers rows
    [base+gw*g, base+gw*(g+1)).  Returns (top_vals [B,K+1], top_idx [B,K+1])."""
    # expand groups to rows
    rows_l, vals_l = [], []
    for jb, (base, w, gw) in enumerate(BLOCKS):
        gidx = idx[:, :, jb * 8:(jb + 1) * 8].astype(np.int64)    # [8, B, 8]
        gval = vals[:, :, jb * 8:(jb + 1) * 8]
        core_off = (np.arange(NCORES, dtype=np.int64) * NSHARD)[:, None, None]
        start = core_off + base + gw * gidx                        # [8, B, 8]
        r = start[..., None] + np.arange(gw, dtype=np.int64)       # [8, B, 8, gw]
        v = np.broadcast_to(gval[..., None], r.shape)
        rows_l.append(np.transpose(r, (1, 0, 2, 3)).reshape(B, -1))
        vals_l.append(np.transpose(v, (1, 0, 2, 3)).reshape(B, -1))
    rows = np.concatenate(rows_l, axis=1)                          # [B, ~12.8k]
    rvals = np.concatenate(vals_l, axis=1)
    # upper bound on the true cos of each candidate row (group max + error)
    ub = (rvals + DELTA_RAW) / mnorm[rows]

    # iterative rescore: exact-rescore in ub-descending chunks until the
    # remaining upper bounds cannot beat the current 4th-best exact value
    order_ub = np.argsort(-ub, axis=1)
    ncand = rows.shape[1]
    CH0, CH = 1024, 1024
    top_idx = np.zeros((B, K + 1), np.int64)
    top_val = np.full((B, K + 1), -2.0, np.float32)
    qnorm = np.linalg.norm  # alias

    def _rescore(q, cand_rows):
        rowsf = img_memory[cand_rows]
        rn = rowsf / qnorm(rowsf, axis=1, keepdims=True)
        return (rn @ qn[q].astype(np.float64)).astype(np.float32)

    for q in range(B):
        oq = order_ub[q]
        done = 0
        best_rows = np.empty(0, np.int64)
        best_sims = np.empty(0, np.float32)
        while done < ncand:
            take = CH0 if done == 0 else CH
            chunk = oq[done:done + take]
            done += take
            cr = rows[q, chunk]
            cs = _rescore(q, cr)
            best_rows = np.concatenate([best_rows, cr])
            best_sims = np.concatenate([best_sims, cs])
            o = np.lexsort((best_rows, -best_sims))[:K + 1]
            best_rows, best_sims = best_rows[o], best_sims[o]
            if done < ncand and ub[q, oq[done]] <= best_sims[K] + 1e-7:
                break
        top_idx[q] = best_rows
        top_val[q] = best_sims
    cand = top_idx  # for the fallback path below

    # ---- containment + tie-duplication checks --------------------------------
    v8 = vals[:, :, 7::8]                                          # [8, B, NBLK]
    minn = np.empty((NCORES, NBLK), np.float32)
    maxn = np.empty((NCORES, NBLK), np.float32)
    for c in range(NCORES):
        for jb, (base, w, _) in enumerate(BLOCKS):
            seg = mnorm[c * NSHARD + base: c * NSHARD + base + w]
            minn[c, jb] = seg.min() * (1 - 1e-5)
            maxn[c, jb] = seg.max() * (1 + 1e-5)
    num = v8 + DELTA_RAW
    denom = np.where(num >= 0, minn[:, None, :], maxn[:, None, :])
    ub = num / denom                                               # [8, B, NBLK]
    tau = top_val[:, K]
    viol = set(map(tuple, np.argwhere(ub > (tau[None, :, None] - 1e-6))))

    # FIND_INDEX8 returns the first match per value: exactly-equal f32 group
    # maxes would alias to one index and silently drop a group.
    iview = idx.reshape(NCORES, B, NBLK, 8)
    for c, q, jb in np.argwhere(
            (np.sort(iview, axis=3)[:, :, :, 1:] ==
             np.sort(iview, axis=3)[:, :, :, :-1]).any(axis=3)):
        viol.add((c, q, jb))

    if viol:
        per_q = {}
        for c, q, jb in viol:
            per_q.setdefault(q, set()).add((c, jb))
        for q, blks in per_q.items():
            extra_idx = []
            for c, jb in blks:
                base, w, _ = BLOCKS[jb]
                lo = c * NSHARD + base
                extra_idx.append(np.arange(lo, lo + w, dtype=np.int64))
            extra_idx = np.concatenate(extra_idx + [cand[q]])
            extra_idx = np.unique(extra_idx)
            rowsf = img_memory[extra_idx]
            rn = rowsf / np.linalg.norm(rowsf, axis=1, keepdims=True)
            s = (rn @ qn[q]).astype(np.float32)
            o = np.lexsort((extra_idx, -s))[:K + 1]
            top_idx[q] = extra_idx[o]
            top_val[q] = s[o]
    return top_val, top_idx


def _assemble(i_feats, t_feats, img_memory, txt_memory, top_val, top_idx):
    dt = np.float32
    cand_vals = top_val[:, 1:].astype(dt)                   # [B, K]
    cand_idx = top_idx[:, 1:]
    valid = cand_vals > ID_THRESHOLD

    neg_inf = np.float32(-1e30)
    logits = np.concatenate(
        [np.full((B, 1), SOFT_SCALE, dt),
         np.where(valid, SOFT_SCALE * cand_vals, neg_inf)], axis=1)
    lm = logits.max(axis=1, keepdims=True)
    e = np.exp(logits - lm)
    w = 1.0 - e / e.sum(axis=1, keepdims=True)
    sample_weight = np.where(valid, w[:, 1:], 0.0).astype(dt)

    safe_idx = np.where(valid, cand_idx, 0)
    m = valid[..., None].astype(dt)
    pos_img = img_memory[safe_idx] * m                      # [B, K, D]
    pos_txt = txt_memory[safe_idx] * m

    new_img = np.concatenate([i_feats, pos_img.reshape(B * K, D)], 0).astype(dt)
    new_txt = np.concatenate([t_feats, pos_txt.reshape(B * K, D)], 0).astype(dt)

    qpid = np.arange(B)
    slot_global = np.arange(B * K).reshape(B, K)
    spid = np.where(valid, qpid[:, None], -(slot_global + 1))
    pid = np.concatenate([qpid, spid.reshape(-1)])
    labels = (pid[:, None] == pid[None, :]).astype(dt)

    soft_block = np.zeros((B, B, K), dt)
    soft_block[qpid, qpid, :] = sample_weight
    top = np.concatenate([np.eye(B, dtype=dt), soft_block.reshape(B, B * K)], 1)
    labels[:B, :] = top

    return np.concatenate([new_img, new_txt, labels], axis=0)


def kernel(i_feats, t_feats, img_memory, txt_memory):
    i_feats = np.asarray(i_feats, dtype=np.float32)
    t_feats = np.asarray(t_feats, dtype=np.float32)
    img_memory = np.asarray(img_memory, dtype=np.float32)
    txt_memory = np.asarray(txt_memory, dtype=np.float32)

    qn, qn_bf, qt_host, memt_cores = _prep_inputs(i_feats, img_memory)
    vals, idx, _ = run_device_topk(qt_host, memt_cores, trace=False)

    mnorm = np.sqrt(np.einsum("nd,nd->n", img_memory, img_memory))
    top_val, top_idx = _exact_topk(qn, img_memory, vals, idx, mnorm)
    return _assemble(i_feats, t_feats, img_memory, txt_memory, top_val, top_idx)


# revision 7
# speedup vs baseline: 1.0934x; 1.0934x over previous
"""Sharded kNN retrieval kernel for Trainium2 (8 NeuronCores).

Strategy (v2, fp8 + grouped top-8):
  - Host: l2-normalize queries; queries bf16, img_memory fp8(e4m3, x16 scale);
    per-core transposed layout memt[c] = [128(d-part), 4(d-block), 25000(rows)].
  - Device (SPMD x8), per 2048-col block:
      PE:  sim = qT.T @ memT  (bf16 x fp8 -> PSUM f32)             ~2.1us
      DVE: 16:1 group max (tensor_reduce) straight off PSUM,        ~2.4us
           then MAX8 + FIND_INDEX8 over the 128 group maxes
      DMA: 1.05 MB fp8 per block                                    ~2.6us
    Stages pipeline; each core streams its 12.8 MB shard once.
  - Host: expand group candidates (x16 rows), rank by approx cos, exact f32
    rescore of the top rows; containment + duplicate-index checks with exact
    block-recompute fallback; assemble the reference output exactly.
"""

import numpy as np
import ml_dtypes

import concourse.bass as bass
import concourse.tile as tile
import concourse.mybir as mybir
from concourse import bass_utils

BF16 = ml_dtypes.bfloat16
FP8 = ml_dtypes.float8_e4m3

B = 128
D = 512
N = 200000
NCORES = 8
NSHARD = N // NCORES          # 25000
K = 3
ID_THRESHOLD = 0.15
SOFT_SCALE = 5.0
MSCALE = np.float32(16.0)     # img_memory scaled by 16 before fp8 quantization

WBLK = 2048
_full = NSHARD // WBLK        # 12
_rem = NSHARD - _full * WBLK  # 424
# (base, width, group_width)
BLOCKS = [(j * WBLK, WBLK, 16) for j in range(_full)] + \
         ([(NSHARD - _rem, _rem, 8)] if _rem else [])
NBLK = len(BLOCKS)            # 13

_NC_CACHE = {}


def _build_nc():
    if "nc" in _NC_CACHE:
        return _NC_CACHE["nc"]
    nc = bass.Bass("TRN2", target_bir_lowering=False, debug=False, num_devices=NCORES)
    qt = nc.dram_tensor("qt", [128, 4, 128], mybir.dt.bfloat16, kind="ExternalInput")
    # packed: block j occupies cols [4*base, 4*base+4*w) with b-major sublayout
    memt = nc.dram_tensor("memt", [128, 4 * NSHARD], mybir.dt.float8e4, kind="ExternalInput")
    vals_out = nc.dram_tensor("vals", [128, NBLK * 8], mybir.dt.float32, kind="ExternalOutput")
    idx_out = nc.dram_tensor("idx", [128, NBLK * 8], mybir.dt.uint16, kind="ExternalOutput")

    with tile.TileContext(nc) as tc:
        with (
            tc.tile_pool(name="qt_pool", bufs=1) as qt_pool,
            tc.tile_pool(name="mem_lo", bufs=3) as mem_lo,
            tc.tile_pool(name="mem_hi", bufs=3) as mem_hi,
            tc.tile_pool(name="red_pool", bufs=2) as red_pool,
            tc.tile_pool(name="res_pool", bufs=1) as res_pool,
            tc.tile_pool(name="psum_pool", bufs=2, space="PSUM") as psum_pool,
        ):
            qt_tile = qt_pool.tile([128, 4, 128], mybir.dt.bfloat16)
            nc.scalar.dma_start(qt_tile[:], qt[:])

            vals_tile = res_pool.tile([128, NBLK * 8], mybir.dt.float32)
            idx_tile = res_pool.tile([128, NBLK * 8], mybir.dt.uint16)

            for j, (base, w, gw) in enumerate(BLOCKS):
                g = w // gw
                # chunk-major block layout: [chunk][b][sw] fp8, chunks of 512
                if w == WBLK:
                    lo = mem_lo.tile([128, 4096], mybir.dt.float8e4, tag="lo")
                    nc.sync.dma_start(lo[:], memt[:, 4 * base:4 * base + 4096])
                    hi = mem_hi.tile([128, 4096], mybir.dt.float8e4, tag="hi")
                    nc.scalar.dma_start(hi[:], memt[:, 4 * base + 4096:4 * base + 8192])
                    parts = [(lo, 0, 0), (lo, 512, 2048), (hi, 1024, 0), (hi, 1536, 2048)]
                else:
                    lo = mem_lo.tile([128, 4 * w], mybir.dt.float8e4, tag="lo")
                    nc.sync.dma_start(lo[:], memt[:, 4 * base:4 * base + 4 * w])
                    parts = [(lo, 0, 0)]
                ps = psum_pool.tile([128, g, gw], mybir.dt.float32, tag="ps")
                for mt, s0, off in parts:
                    sw = min(512, w - s0)
                    for b in range(4):
                        nc.tensor.matmul(
                            ps[:, s0 // gw:(s0 + sw) // gw, :],
                            qt_tile[:, b, :],
                            mt[:, off + b * sw:off + (b + 1) * sw],
                            start=(b == 0),
                            stop=(b == 3),
                        )
                red = red_pool.tile([128, g], mybir.dt.float32, tag="red")
                nc.vector.tensor_reduce(red[:], ps[:], axis=mybir.AxisListType.X,
                                        op=mybir.AluOpType.max)
                nc.vector.max(vals_tile[:, j * 8:(j + 1) * 8], red[:])
                nc.vector.max_index(idx_tile[:, j * 8:(j + 1) * 8],
                                    vals_tile[:, j * 8:(j + 1) * 8], red[:])

            nc.sync.dma_start(vals_out[:], vals_tile[:])
            nc.scalar.dma_start(idx_out[:], idx_tile[:])
    _split_excess_waits(nc)
    _NC_CACHE["nc"] = nc
    return nc


def _split_excess_waits(nc, keep=1):
    """Walrus's MM instruction struct fits only one embedded sync wait; move
    extra waits emitted by Tile onto standalone NoOps just before the MM."""
    ctr = 0
    for fn in nc.m.functions:
        for blk in fn.blocks:
            newl = []
            for inst in blk.instructions:
                si = inst.sync_info
                if (type(inst).__name__ != "InstNoOp" and si is not None
                        and si.on_wait and len(si.on_wait) > keep):
                    waits = list(si.on_wait)
                    for w in waits[:-keep]:
                        nop = mybir.InstNoOp(name=f"I-waitnop-{ctr}")
                        ctr += 1
                        nop.engine = inst.engine
                        nop.sync_info = mybir.SyncInfo(on_wait=[w], on_update=[])
                        newl.append(nop)
                    inst.sync_info = mybir.SyncInfo(
                        on_wait=waits[-keep:], on_update=list(si.on_update or []))
                newl.append(inst)
            blk.instructions = newl


def run_device_topk(qt_host, memt_cores, trace=False):
    """Run the SPMD device kernel.  Returns (vals [8,128,NBLK*8] f32 raw-dot,
    idx [8,128,NBLK*8] uint32 group indices, BassKernelResults)."""
    nc = _build_nc()
    in_maps = [{"qt": qt_host, "memt": memt_cores[c]} for c in range(NCORES)]
    res = bass_utils.run_bass_kernel_spmd(
        nc, in_maps, core_ids=list(range(NCORES)), trace=trace,
    )
    vals = np.stack([res.results[c]["vals"] for c in range(NCORES)]) / MSCALE
    idx = np.stack([res.results[c]["idx"] for c in range(NCORES)])
    return vals, idx, res


def _prep_inputs(i_feats, img_memory):
    qn = i_feats / np.linalg.norm(i_feats, axis=1, keepdims=True)
    qn = qn.astype(np.float32)
    qn_bf = qn.astype(BF16)
    qt_host = np.ascontiguousarray(qn_bf.reshape(B, 4, 128).transpose(2, 1, 0))

    def _pack(c):
        shard8 = (img_memory[c * NSHARD:(c + 1) * NSHARD] * MSCALE).astype(FP8)
        segs = []
        for base, w, _ in BLOCKS:
            for s0 in range(0, w, 512):         # chunk-major within each block
                sw = min(512, w - s0)
                seg = shard8[base + s0:base + s0 + sw].reshape(sw, 4, 128)
                segs.append(np.ascontiguousarray(seg.transpose(2, 1, 0)).reshape(128, 4 * sw))
        return np.concatenate(segs, axis=1)

    from concurrent.futures import ThreadPoolExecutor
    with ThreadPoolExecutor(max_workers=NCORES) as ex:
        memt_cores = list(ex.map(_pack, range(NCORES)))
    return qn, qn_bf, qt_host, memt_cores


# max |device_raw/16 - exact_raw| bound: fp8 quant (~0.13 observed max) +
# bf16 query rounding (2^-8*||m|| ~ 0.09) + f32 accum slack.
DELTA_RAW = np.float32(0.35)


def _exact_topk(qn, img_memory, vals, idx, mnorm):
    """Global exact top-(K+1) per query from device group candidates.

    Device reports top-8 *groups* per block: group g covers rows
    [base+gw*g, base+gw*(g+1)).  Returns (top_vals [B,K+1], top_idx [B,K+1])."""
    # expand groups to rows
    rows_l, vals_l = [], []
    for jb, (base, w, gw) in enumerate(BLOCKS):
        gidx = idx[:, :, jb * 8:(jb + 1) * 8].astype(np.int64)    # [8, B, 8]
        gval = vals[:, :, jb * 8:(jb + 1) * 8]
        core_off = (np.arange(NCORES, dtype=np.int64) * NSHARD)[:, None, None]
        start = core_off + base + gw * gidx                        # [8, B, 8]
        r = start[..., None] + np.arange(gw, dtype=np.int64)       # [8, B, 8, gw]
        v = np.broadcast_to(gval[..., None], r.shape)
        rows_l.append(np.transpose(r, (1, 0, 2, 3)).reshape(B, -1))
        vals_l.append(np.transpose(v, (1, 0, 2, 3)).reshape(B, -1))
    rows = np.concatenate(rows_l, axis=1)                          # [B, ~12.8k]
    rvals = np.concatenate(vals_l, axis=1)
    # upper bound on the true cos of each candidate row (group max + error)
    ub = (rvals + DELTA_RAW) / mnorm[rows]

    # iterative rescore: exact-rescore in ub-descending chunks until the
    # remaining upper bounds cannot beat the current 4th-best exact value
    order_ub = np.argsort(-ub, axis=1)
    ncand = rows.shape[1]
    CH0, CH = 1024, 1024
    top_idx = np.zeros((B, K + 1), np.int64)
    top_val = np.full((B, K + 1), -2.0, np.float32)
    qnorm = np.linalg.norm  # alias

    def _rescore(q, cand_rows):
        rowsf = img_memory[cand_rows]
        rn = rowsf / qnorm(rowsf, axis=1, keepdims=True)
        return (rn @ qn[q].astype(np.float64)).astype(np.float32)

    for q in range(B):
        oq = order_ub[q]
        done = 0
        best_rows = np.empty(0, np.int64)
        best_sims = np.empty(0, np.float32)
        while done < ncand:
            take = CH0 if done == 0 else CH
            chunk = oq[done:done + take]
            done += take
            cr = rows[q, chunk]
            cs = _rescore(q, cr)
            best_rows = np.concatenate([best_rows, cr])
            best_sims = np.concatenate([best_sims, cs])
            o = np.lexsort((best_rows, -best_sims))[:K + 1]
            best_rows, best_sims = best_rows[o], best_sims[o]
            if done < ncand and ub[q, oq[done]] <= best_sims[K] + 1e-7:
                break
        top_idx[q] = best_rows
        top_val[q] = best_sims
    cand = top_idx  # for the fallback path below

    # ---- containment + tie-duplication checks --------------------------------
    v8 = vals[:, :, 7::8]                                          # [8, B, NBLK]
    minn = np.empty((NCORES, NBLK), np.float32)
    maxn = np.empty((NCORES, NBLK), np.float32)
    for c in range(NCORES):
        for jb, (base, w, _) in enumerate(BLOCKS):
            seg = mnorm[c * NSHARD + base: c * NSHARD + base + w]
            minn[c, jb] = seg.min() * (1 - 1e-5)
            maxn[c, jb] = seg.max() * (1 + 1e-5)
    num = v8 + DELTA_RAW
    denom = np.where(num >= 0, minn[:, None, :], maxn[:, None, :])
    ub = num / denom                                               # [8, B, NBLK]
    tau = top_val[:, K]
    viol = set(map(tuple, np.argwhere(ub > (tau[None, :, None] - 1e-6))))

    # FIND_INDEX8 returns the first match per value: exactly-equal f32 group
    # maxes would alias to one index and silently drop a group.
    iview = idx.reshape(NCORES, B, NBLK, 8)
    for c, q, jb in np.argwhere(
            (np.sort(iview, axis=3)[:, :, :, 1:] ==
             np.sort(iview, axis=3)[:, :, :, :-1]).any(axis=3)):
        viol.add((c, q, jb))

    if viol:
        per_q = {}
        for c, q, jb in viol:
            per_q.setdefault(q, set()).add((c, jb))
        for q, blks in per_q.items():
            extra_idx = []
            for c, jb in blks:
                base, w, _ = BLOCKS[jb]
                lo = c * NSHARD + base
                extra_idx.append(np.arange(lo, lo + w, dtype=np.int64))
            extra_idx = np.concatenate(extra_idx + [cand[q]])
            extra_idx = np.unique(extra_idx)
            rowsf = img_memory[extra_idx]
            rn = rowsf / np.linalg.norm(rowsf, axis=1, keepdims=True)
            s = (rn @ qn[q]).astype(np.float32)
            o = np.lexsort((extra_idx, -s))[:K + 1]
            top_idx[q] = extra_idx[o]
            top_val[q] = s[o]
    return top_val, top_idx


def _assemble(i_feats, t_feats, img_memory, txt_memory, top_val, top_idx):
    dt = np.float32
    cand_vals = top_val[:, 1:].astype(dt)                   # [B, K]
    cand_idx = top_idx[:, 1:]
    valid = cand_vals > ID_THRESHOLD

    neg_inf = np.float32(-1e30)
    logits = np.concatenate(
        [np.full((B, 1), SOFT_SCALE, dt),
         np.where(valid, SOFT_SCALE * cand_vals, neg_inf)], axis=1)
    lm = logits.max(axis=1, keepdims=True)
    e = np.exp(logits - lm)
    w = 1.0 - e / e.sum(axis=1, keepdims=True)
    sample_weight = np.where(valid, w[:, 1:], 0.0).astype(dt)

    safe_idx = np.where(valid, cand_idx, 0)
    m = valid[..., None].astype(dt)
    pos_img = img_memory[safe_idx] * m                      # [B, K, D]
    pos_txt = txt_memory[safe_idx] * m

    new_img = np.concatenate([i_feats, pos_img.reshape(B * K, D)], 0).astype(dt)
    new_txt = np.concatenate([t_feats, pos_txt.reshape(B * K, D)], 0).astype(dt)

    qpid = np.arange(B)
    slot_global = np.arange(B * K).reshape(B, K)
    spid = np.where(valid, qpid[:, None], -(slot_global + 1))
    pid = np.concatenate([qpid, spid.reshape(-1)])
    labels = (pid[:, None] == pid[None, :]).astype(dt)

    soft_block = np.zeros((B, B, K), dt)
    soft_block[qpid, qpid, :] = sample_weight
    top = np.concatenate([np.eye(B, dtype=dt), soft_block.reshape(B, B * K)], 1)
    labels[:B, :] = top

    return np.concatenate([new_img, new_txt, labels], axis=0)


def kernel(i_feats, t_feats, img_memory, txt_memory):
    i_feats = np.asarray(i_feats, dtype=np.float32)
    t_feats = np.asarray(t_feats, dtype=np.float32)
    img_memory = np.asarray(img_memory, dtype=np.float32)
    txt_memory = np.asarray(txt_memory, dtype=np.float32)

    qn, qn_bf, qt_host, memt_cores = _prep_inputs(i_feats, img_memory)
    vals, idx, _ = run_device_topk(qt_host, memt_cores, trace=False)

    mnorm = np.sqrt(np.einsum("nd,nd->n", img_memory, img_memory))
    top_val, top_idx = _exact_topk(qn, img_memory, vals, idx, mnorm)
    return _assemble(i_feats, t_feats, img_memory, txt_memory, top_val, top_idx)
